# revision 12
# baseline (speedup 1.0000x reference)
"""Batched triu-scatter kernel for Trainium2.

x: [64, 2098176] f32 (packed upper-triangular rows of a 2048x2048 matrix)
-> out: [64, 2048, 2048] f32 with x scattered into the upper triangle,
zeros below the diagonal.

Distribution: row-interleaved across the 8 NeuronCores — core k handles
matrix rows r = k + 8*i (i = 0..255) of ALL 64 samples. Slots are merged
into 16 groups of 16 consecutive slots; group g is one 3-level-AP DMA of
[B=64, 16 slots, W_g] with W_g = 2048-128g: every row in the group is
left-padded with zeros to width W_g (pad = 8j+k for slot 16g+j on core
k), which lands on legitimately-zero cells left of the diagonal. Cells
further left are never written; run_bass_kernel_spmd pre-zeroes and
donates ExternalOutput buffers. The pad costs ~5.8% extra traffic but
reduces the program to 16 DMA instructions, each with 1024 descriptors
and its own dedicated offset registers.

Why dynamic offsets at all: NTFF traces show the chip's DRAM channels
are shared by core pairs ((0,1),(2,3),(4,5),(6,7) by q_dram_channel),
and pair partners execute this identical SPMD program in lockstep
through identically laid out buffers. In roughly half of runs one pair
spends most of the run at HALF DMA rate (~165 vs ~330 GB/s per core,
and the reported time is the max over cores) — consistent with the two
streams colliding in DRAM bank geometry when their base addresses are
congruent. Every DMA offset therefore adds partition_id * PHASE (odd
multiples of 512B, nonzero mod every power of two >= 1KB), permanently
decorrelating pair partners regardless of allocator luck. Each DMA gets
its own registers, written once before issue — the DGE reads registers
asynchronously, so reusing one register across in-flight DMAs races
(observed as a scrambled output).

The 32 tensors per core (one src + one dst per group) also spread the
traffic over 32 independent allocations, and the two DGE queues are fed
big and small groups alternately so descriptor sizes stay in the
efficient 2-8KB band throughout the run.

Net HBM traffic per core: ~71MB read + ~71MB written.
"""

import os
import time

import numpy as np

import concourse.bass as bass
import concourse.mybir as mybir
from concourse.bass_utils import run_bass_kernel_spmd

_VERBOSE = bool(os.environ.get("KERNEL_VERBOSE"))


def _log(msg):
    if _VERBOSE:
        print(f"[kernel +{time.time() - _T0:.1f}s] {msg}", flush=True)


_T0 = time.time()

M = 2048
NT = M * (M + 1) // 2  # 2098176
B = 64
N_CORES = 8
NSLOTS = M // N_CORES  # 256
NGROUP = 16
GW = NSLOTS // NGROUP  # 16 slots per group
WIDTH = [M - 128 * g for g in range(NGROUP)]  # group slot width

# Per-core phase (elements): odd multiples of 512B so pair partners'
# addresses differ mod every power-of-two DRAM interleave >= 1KB.
PH_SRC = 2176  # 8704 B
PH_DST = 2688  # 10752 B
# Per-tensor static lead pads decorrelate a core's own 32 streams.
SPAD_S = [g * 1088 for g in range(NGROUP)]
SPAD_D = [g * 1344 for g in range(NGROUP)]
SRC_TSIZE = [SPAD_S[g] + (N_CORES - 1) * PH_SRC + GW * B * WIDTH[g] for g in range(NGROUP)]
DST_TSIZE = [SPAD_D[g] + (N_CORES - 1) * PH_DST + GW * B * M for g in range(NGROUP)]

# queue schedules: blend big and small groups on each queue
RING_GROUPS = {0: [0, 14, 2, 12, 4, 10, 6, 8], 1: [1, 15, 3, 13, 5, 11, 7, 9]}

_nc_cache = None
_NEFF_CACHE_DIR = os.path.expanduser("~/.cache/bass_neff_cache")


def _install_neff_cache():
    """Wrap bass2jax's compile_bir_kernel with a content-addressed disk
    cache so repeat runs of this (deterministic) program skip the
    multi-minute walrus compile."""
    import hashlib
    import shutil as _sh

    import concourse.bass2jax as b2j

    if getattr(b2j.compile_bir_kernel, "_is_neff_cache", False):
        return
    orig = b2j.compile_bir_kernel

    def cached(bir_json, tmpdir, neff_name="file.neff"):
        key = hashlib.sha256(
            bir_json if isinstance(bir_json, bytes) else bir_json.encode()
        ).hexdigest()
        cpath = os.path.join(_NEFF_CACHE_DIR, f"{key}.neff")
        dst = os.path.join(tmpdir, neff_name)
        if os.path.exists(cpath):
            _sh.copy(cpath, dst)
            _log(f"NEFF cache hit {key[:12]}")
            return dst
        neff = orig(bir_json, tmpdir, neff_name)
        try:
            os.makedirs(_NEFF_CACHE_DIR, exist_ok=True)
            _sh.copy(neff, cpath + ".tmp")
            os.replace(cpath + ".tmp", cpath)
        except OSError:
            pass
        return neff

    cached._is_neff_cache = True
    b2j.compile_bir_kernel = cached


def _build():
    nc = bass.Bass(num_devices=N_CORES)
    xs = [
        nc.dram_tensor(f"x{g}", [SRC_TSIZE[g]], mybir.dt.float32, kind="ExternalInput")
        for g in range(NGROUP)
    ]
    ys = [
        nc.dram_tensor(f"y{g}", [DST_TSIZE[g]], mybir.dt.float32, kind="ExternalOutput")
        for g in range(NGROUP)
    ]
    with nc.semaphore("sem_a") as sem_a, nc.semaphore("sem_b") as sem_b:
        sems = {0: sem_a, 1: sem_b}
        engs = {0: nc.sync, 1: nc.scalar}
        for ring in (0, 1):
            e = engs[ring]
            r_pid = e.alloc_register(f"pid_{ring}")
            e.reg_load(r_pid, nc.partition_id_tensor[0:1, 0:1])
            r_phs = e.alloc_register(f"phs_{ring}")
            e.reg_mul(r_phs, r_pid, PH_SRC)
            r_phd = e.alloc_register(f"phd_{ring}")
            e.reg_mul(r_phd, r_pid, PH_DST)
            e.free_register(r_pid)
            for g in RING_GROUPS[ring]:
                w = WIDTH[g]
                so = SPAD_S[g]
                do = SPAD_D[g] + (M - w)
                # Dedicated registers per DMA, written once before issue;
                # donate=True hands ownership to the snap (no copy reg).
                r_s = e.alloc_register(f"s{g}")
                e.reg_add(r_s, r_phs, so)
                r_d = e.alloc_register(f"d{g}")
                e.reg_add(r_d, r_phd, do)
                src = bass.AP(
                    xs[g][:].tensor,
                    e.snap(
                        r_s, donate=True,
                        min_val=so, max_val=so + (N_CORES - 1) * PH_SRC,
                    ),
                    [[w, B], [B * w, GW], [1, w]],
                )
                dst = bass.AP(
                    ys[g][:].tensor,
                    e.snap(
                        r_d, donate=True,
                        min_val=do, max_val=do + (N_CORES - 1) * PH_DST,
                    ),
                    [[M, B], [B * M, GW], [1, w]],
                )
                e.dma_start(dst, src).then_inc(sems[ring], 16)
        nc.sync.wait_ge(sem_a, 16 * len(RING_GROUPS[0]))
        nc.scalar.wait_ge(sem_b, 16 * len(RING_GROUPS[1]))
    return nc


def _get_nc():
    global _nc_cache
    if _nc_cache is None:
        _nc_cache = _build()
    return _nc_cache


def _pack_core(x, k):
    """Pack core k's inputs: group g slot j holds [64, W_g] =
    [(8j+k) zeros || row k+8(16g+j)], at the core's phase offset."""
    out = {}
    for g in range(NGROUP):
        w = WIDTH[g]
        xk = np.zeros((SRC_TSIZE[g],), np.float32)
        base = SPAD_S[g] + k * PH_SRC
        blk = xk[base : base + GW * B * w].reshape(GW, B, w)
        for j in range(GW):
            r = k + 8 * (16 * g + j)
            L = M - r
            o = r * M - r * (r - 1) // 2  # packed triu row offset
            blk[j, :, 8 * j + k :] = x[:, o : o + L]
        out[f"x{g}"] = xk
    return out


def kernel(x: np.ndarray, _trace: bool = False):
    assert x.shape == (B, NT), x.shape
    global _T0
    _T0 = time.time()
    x = np.ascontiguousarray(x, dtype=np.float32)
    _log("input ready")
    _install_neff_cache()
    nc = _get_nc()
    _log("nc built")
    in_maps = [_pack_core(x, k) for k in range(N_CORES)]
    _log("packed")
    # The first execution after an unclean device state occasionally fails
    # with NRT_EXEC_UNIT_UNRECOVERABLE; a retry on a re-initialized device
    # succeeds, so try up to 3 times.
    last_exc = None
    for _attempt in range(3):
        try:
            res = run_bass_kernel_spmd(
                nc, in_maps, core_ids=list(range(N_CORES)), trace=_trace
            )
            break
        except Exception as e:  # noqa: BLE001
            _log(f"attempt {_attempt} failed: {type(e).__name__}: {e}")
            last_exc = e
    else:
        raise last_exc
    _log("executed")
    out = np.empty((B, M, M), np.float32)
    for k in range(N_CORES):
        rk = res.results[k]
        for g in range(NGROUP):
            base = SPAD_D[g] + k * PH_DST
            blk = rk[f"y{g}"][base : base + GW * B * M].reshape(GW, B, M)
            for j in range(GW):
                out[:, k + 8 * (16 * g + j), :] = blk[j]
    _log("reassembled")
    if _trace:
        return out, res
    return out


# revision 13
# speedup vs baseline: 1.3828x; 1.3828x over previous
"""Batched triu-scatter kernel for Trainium2.

x: [64, 2098176] f32 (packed upper-triangular rows of a 2048x2048 matrix)
-> out: [64, 2048, 2048] f32 with x scattered into the upper triangle,
zeros below the diagonal.

Distribution: row-interleaved across the 8 NeuronCores — core k handles
matrix rows r = k + 8*i (i = 0..255) of ALL 64 samples, so the per-DMA
outer dim is 64 (engages all 16 SDMA engines). Host-side packing gives
every core an IDENTICAL program (required for SPMD): slot i is padded to
S_i = 2048 - 8*i = L + k elements (k zeros up front) so access patterns
don't depend on k; the pad zeros land on legitimately-zero cells left of
the diagonal, and everything further left is never written
(run_bass_kernel_spmd pre-zeroes + donates ExternalOutput buffers).

Measures against behaviors observed in NTFF traces:

1. Slots are issued in complement pairs (p, 255-p), pairs alternating
   between the two DGE queues. Each pair moves a constant 526KB, so both
   queues carry equal bytes over time and the engines always see a blend
   of large (up to 8KB) and small descriptors — eliminating the pure
   tiny-packet tail phase that ran at <half rate.

2. The per-core input is split into 8 ExternalInput tensors and the
   output into 8 ExternalOutput tensors (slot i -> src i%8, dst
   (i//8)%8): 16 independent allocations per core stripe the traffic
   across placement domains.

3. Core-parity schedule reversal: cores (0,1), (2,3), (4,5), (6,7)
   share a DRAM channel pairwise (q_dram_channel in the NTFF), and in
   roughly half of runs one pair spends most of the run with one core at
   full DMA rate (~330 GB/s) and the other at half (~165 GB/s) — the
   reported time is the max over cores. Pair partners otherwise execute
   this identical SPMD program in lockstep through identically laid out
   buffers, maximizing stream collision. A branch on partition_id parity
   makes even cores emit the DMA sequence forward and odd cores emit it
   in reverse, so partners always work on different regions with
   different descriptor-size mixes. The branch bodies contain only
   statically-addressed DMAs (dynamic per-DMA offsets would exhaust the
   engine's ~54 registers and are read asynchronously by the DGE).

Net HBM traffic per core: 67MB read + 67MB written.
"""

import os
import time

import numpy as np

import concourse.bass as bass
import concourse.mybir as mybir
from concourse.bass_utils import run_bass_kernel_spmd

_VERBOSE = bool(os.environ.get("KERNEL_VERBOSE"))


def _log(msg):
    if _VERBOSE:
        print(f"[kernel +{time.time() - _T0:.1f}s] {msg}", flush=True)


_T0 = time.time()

M = 2048
NT = M * (M + 1) // 2  # 2098176
B = 64
N_CORES = 8
NSLOTS = M // N_CORES  # 256
NSPLIT = 8  # src/dst tensors per core
S = [M - 8 * i for i in range(NSLOTS)]  # slot widths (same for all cores)

SRC_OF = [i % NSPLIT for i in range(NSLOTS)]
DST_OF = [(i // 8) % NSPLIT for i in range(NSLOTS)]
SRC_SLOTS = [[i for i in range(NSLOTS) if SRC_OF[i] == s] for s in range(NSPLIT)]
DST_SLOTS = [[i for i in range(NSLOTS) if DST_OF[i] == d] for d in range(NSPLIT)]
# element offset of slot i inside its src tensor (after the static pad)
SRC_OFF = {}
for s in range(NSPLIT):
    o = 0
    for i in SRC_SLOTS[s]:
        SRC_OFF[i] = o
        o += B * S[i]
SRC_SIZE = [sum(B * S[i] for i in SRC_SLOTS[s]) for s in range(NSPLIT)]
DST_POS = {i: p for d in range(NSPLIT) for p, i in enumerate(DST_SLOTS[d])}
N_DST = NSLOTS // NSPLIT  # 32 slots per dst tensor

# Static per-tensor lead pads vary allocation sizes/offsets a little.
SPAD_SRC = [s * 1088 for s in range(NSPLIT)]
SPAD_DST = [d * 1344 for d in range(NSPLIT)]
SRC_TSIZE = [SPAD_SRC[s] + SRC_SIZE[s] for s in range(NSPLIT)]
DST_TSIZE = [SPAD_DST[d] + N_DST * B * M for d in range(NSPLIT)]

_nc_cache = None
_NEFF_CACHE_DIR = os.path.expanduser("~/.cache/bass_neff_cache")


def _install_neff_cache():
    """Wrap bass2jax's compile_bir_kernel with a content-addressed disk
    cache so repeat runs of this (deterministic) program skip the
    multi-minute walrus compile."""
    import hashlib
    import shutil as _sh

    import concourse.bass2jax as b2j

    if getattr(b2j.compile_bir_kernel, "_is_neff_cache", False):
        return
    orig = b2j.compile_bir_kernel

    def cached(bir_json, tmpdir, neff_name="file.neff"):
        key = hashlib.sha256(
            bir_json if isinstance(bir_json, bytes) else bir_json.encode()
        ).hexdigest()
        cpath = os.path.join(_NEFF_CACHE_DIR, f"{key}.neff")
        dst = os.path.join(tmpdir, neff_name)
        if os.path.exists(cpath):
            _sh.copy(cpath, dst)
            _log(f"NEFF cache hit {key[:12]}")
            return dst
        neff = orig(bir_json, tmpdir, neff_name)
        try:
            os.makedirs(_NEFF_CACHE_DIR, exist_ok=True)
            _sh.copy(neff, cpath + ".tmp")
            os.replace(cpath + ".tmp", cpath)
        except OSError:
            pass
        return neff

    cached._is_neff_cache = True
    b2j.compile_bir_kernel = cached


def _build():
    nc = bass.Bass(num_devices=N_CORES)
    xs = [
        nc.dram_tensor(f"x{s}", [SRC_TSIZE[s]], mybir.dt.float32, kind="ExternalInput")
        for s in range(NSPLIT)
    ]
    ys = [
        nc.dram_tensor(f"y{d}", [DST_TSIZE[d]], mybir.dt.float32, kind="ExternalOutput")
        for d in range(NSPLIT)
    ]
    with nc.semaphore("sem_a") as sem_a, nc.semaphore("sem_b") as sem_b:
        sems = {0: sem_a, 1: sem_b}
        engs = {0: nc.sync, 1: nc.scalar}

        def ap_pair(i):
            w = S[i]
            src = bass.AP(
                xs[SRC_OF[i]][:].tensor,
                SPAD_SRC[SRC_OF[i]] + SRC_OFF[i],
                [[w, B], [1, w]],
            )
            dst = bass.AP(
                ys[DST_OF[i]][:].tensor,
                SPAD_DST[DST_OF[i]] + DST_POS[i] * B * M + (M - w),
                [[M, B], [1, w]],
            )
            return dst, src

        # queue schedules: complement pairs, constant 526KB per pair
        ring_slots = {0: [], 1: []}
        for p in range(NSLOTS // 2):
            ring_slots[p % 2] += [p, NSLOTS - 1 - p]

        for ring in (0, 1):
            e = engs[ring]
            slots = ring_slots[ring]
            r = e.alloc_register(f"par_{ring}")
            e.reg_load(r, nc.partition_id_tensor[0:1, 0:1])
            e.reg_mod(r, r, 2)
            with e.If(e.snap(r, donate=True, min_val=0, max_val=1) == 0):
                for i in slots:
                    e.dma_start(*ap_pair(i)).then_inc(sems[ring], 16)
            with e.Else():
                for i in reversed(slots):
                    e.dma_start(*ap_pair(i)).then_inc(sems[ring], 16)
            e.wait_ge(sems[ring], 16 * len(slots))
    return nc


def _get_nc():
    global _nc_cache
    if _nc_cache is None:
        _nc_cache = _build()
    return _nc_cache


def _pack_core(x, k):
    """Pack core k's inputs: slot i holds [64, S_i] = [k zeros || row k+8i],
    distributed over NSPLIT src arrays."""
    out = {}
    for s in range(NSPLIT):
        xk = np.zeros((SRC_TSIZE[s],), np.float32)
        for i in SRC_SLOTS[s]:
            r = k + 8 * i
            L = M - r
            b0 = SPAD_SRC[s] + SRC_OFF[i]
            seg = xk[b0 : b0 + B * S[i]].reshape(B, S[i])
            o = r * M - r * (r - 1) // 2  # packed triu row offset
            seg[:, k:] = x[:, o : o + L]
        out[f"x{s}"] = xk
    return out


def kernel(x: np.ndarray, _trace: bool = False):
    assert x.shape == (B, NT), x.shape
    global _T0
    _T0 = time.time()
    x = np.ascontiguousarray(x, dtype=np.float32)
    _log("input ready")
    _install_neff_cache()
    nc = _get_nc()
    _log("nc built")
    in_maps = [_pack_core(x, k) for k in range(N_CORES)]
    _log("packed")
    # The first execution after an unclean device state occasionally fails
    # with NRT_EXEC_UNIT_UNRECOVERABLE; a retry on a re-initialized device
    # succeeds, so try up to 3 times.
    last_exc = None
    for _attempt in range(3):
        try:
            res = run_bass_kernel_spmd(
                nc, in_maps, core_ids=list(range(N_CORES)), trace=_trace
            )
            break
        except Exception as e:  # noqa: BLE001
            _log(f"attempt {_attempt} failed: {type(e).__name__}: {e}")
            last_exc = e
    else:
        raise last_exc
    _log("executed")
    out = np.empty((B, M, M), np.float32)
    for k in range(N_CORES):
        rk = res.results[k]
        for d in range(NSPLIT):
            yd = rk[f"y{d}"]  # flat [DST_TSIZE[d]]
            for p, i in enumerate(DST_SLOTS[d]):
                b0 = SPAD_DST[d] + p * B * M
                out[:, k + 8 * i, :] = yd[b0 : b0 + B * M].reshape(B, M)
    _log("reassembled")
    if _trace:
        return out, res
    return out
